# revision 6
# baseline (speedup 1.0000x reference)
"""Causal self-attention TRN2 Bass kernel (v2: fused pipeline + exp split).

Problem: B=2, T=4096, D_MODEL=512, N_HEADS=8, HEAD_DIM=64 (fp32).

Sharding (tensor+data parallel): 8 cores = 2 batches x 4 head-pairs.
Core c handles batch b = c//4 and heads (2g, 2g+1) with g = c%4, over the
full sequence. Each core computes a full-shape [T, 512] partial output
(its two heads' contribution through W_O); the host sums 4 partials per
batch ("unshard" of the tensor-parallel contraction).

v2 structure (single fused loop, one 512-token super-chunk per step):
  per tcx: QKV projection for tokens [tcx*512,(tcx+1)*512) immediately
  followed by attention for query chunk Q=tcx over key chunks 0..4*tcx+3.
  The PE therefore never drains between "phases"; x tile DMA for step
  tcx+1 overlaps the attention of step tcx.

exp is the phase-2 pacer in v1 (one ScalarE ACTIVATE per 128x1024 block,
back-to-back for the whole phase). v2 splits it:
  - diagonal blocks: exact exp on ScalarE + multiplicative causal mask
    on GpSimd (as v1).
  - off-diagonal blocks: alternating between ScalarE (exact exp) and
    DVE using the Schraudolph bit trick: int16(round(x*A + B)) with
    A = 128*log2(e)/8, B = 127*128 - 5.5 gives the bf16 BITS of
    ~exp(x/8) (max rel err ~3.3%, zero-mean; softmax renormalization
    cancels it below the bf16 noise floor -- verified vs reference).
    HW-verified: DVE fp32->int16 conversion rounds-to-nearest and
    saturates (never wraps), and saturation at -32768 = bf16 -0.0 is a
    free "exp(-inf)=0".
Other element-wise rebalancing: psqk->SBUF copy is one FD=1024 ScalarE
op, V copy one FD=512 DVE op (4-D AP), attention-out normalize mults on
GpSimd, output PSUM->SBUF copies alternate ScalarE/DVE.
"""

import math

import ml_dtypes
import numpy as np

import concourse.bass as bass
import concourse.mybir as mybir
import concourse.tile as tile
from concourse.tile import add_dep_helper
from concourse import bacc
from concourse.bass import ds, ts
from concourse.bass_utils import run_bass_kernel_spmd

FP32 = mybir.dt.float32
FP32R = mybir.dt.float32r
BF16 = mybir.dt.bfloat16
I16 = mybir.dt.int16
AF = mybir.ActivationFunctionType

T = 4096
DM = 512
QC = 512  # query-chunk width (free dim)
KC = 128  # key-chunk width (partition dim)

# Schraudolph constants: int16 bits of bf16(exp(x/8))
EXP_A = 128.0 * math.log2(math.e) / 8.0
EXP_B = 127.0 * 128.0 - 5.5

# knobs (test.py can flip before calling kernel())
SCALAR_SHARE = 5  # off-diag block K goes to ScalarE when (K % SCALAR_MOD) < this
SCALAR_MOD = 12
TRACE = False
LAST_RESULTS = None


def build_program(t=T):
    assert t % QC == 0
    nq = t // QC
    nkc = t // KC
    nc = bacc.Bacc("TRN2", target_bir_lowering=False, debug=False)

    xT = nc.dram_tensor("xT", [DM, t], BF16, kind="ExternalInput").ap()
    wq = nc.dram_tensor("wq", [DM, 128], BF16, kind="ExternalInput").ap()
    wk = nc.dram_tensor("wk", [DM, 128], BF16, kind="ExternalInput").ap()
    wv = nc.dram_tensor("wv", [DM, 128], BF16, kind="ExternalInput").ap()
    woT = nc.dram_tensor("woT", [128, DM], BF16, kind="ExternalInput").ap()
    outp = nc.dram_tensor("outp", [t, DM], FP32, kind="ExternalOutput").ap()

    inv_sqrt_d = 1.0 / math.sqrt(64.0)

    with tile.TileContext(nc) as tc:
        with (
            tc.tile_pool(name="consts", bufs=1) as cpool,
            tc.tile_pool(name="persist", bufs=1) as ppool,
            tc.tile_pool(name="xtl", bufs=2) as xpool,
            tc.tile_pool(name="work", bufs=3) as wpool,
            tc.tile_pool(name="ps_sc", bufs=2, space="PSUM") as ps_sc,
            tc.tile_pool(name="ps_pv", bufs=1, space="PSUM") as ps_pv,
            tc.tile_pool(name="ps_mi", bufs=2, space="PSUM") as ps_mi,
        ):
            # ---- constants ----
            wq_s = cpool.tile([128, 512], BF16, name="wq_s")
            wk_s = cpool.tile([128, 512], BF16, name="wk_s")
            wv_s = cpool.tile([128, 512], BF16, name="wv_s")
            woT_s = cpool.tile([128, 512], BF16, name="woT_s")
            nc.sync.dma_start(
                wq_s[:].rearrange("p (d c) -> p d c", d=4),
                wq.rearrange("(d p) c -> p d c", p=128),
            )
            nc.sync.dma_start(
                wk_s[:].rearrange("p (d c) -> p d c", d=4),
                wk.rearrange("(d p) c -> p d c", p=128),
            )
            nc.sync.dma_start(
                wv_s[:].rearrange("p (d c) -> p d c", d=4),
                wv.rearrange("(d p) c -> p d c", p=128),
            )
            nc.sync.dma_start(woT_s[:], woT[:])

            # multiplicative causal mask for diagonal blocks of P^T [k, q]:
            # 1 where k <= q, 0 elsewhere (applied to exp output on GpSimd)
            mask_s = cpool.tile([128, 128], BF16, name="mask_s")
            nc.gpsimd.memset(mask_s[:], 0.0)
            nc.gpsimd.affine_select(
                out=mask_s[:],
                in_=mask_s[:],
                compare_op=mybir.AluOpType.is_gt,
                fill=1.0,
                base=0,
                pattern=[[-1, 128]],
                channel_multiplier=1,
            )

            # ones row at partition 64 for the K=1 reciprocal broadcast
            ones_row = cpool.tile([65, 64], FP32R, name="ones_row")
            nc.vector.memset(ones_row[:].bitcast(FP32), 1.0)

            # ---- persistent activations ----
            # qkT packed in one tile: [:, 0, :] = qT, [:, 1, :] = kT
            # (partitions 0:64 head0 dims, 64:128 head1)
            qkT_s = ppool.tile([128, 2 * t], BF16, name="qkT_s")
            qkT3 = qkT_s[:].rearrange("p (h t) -> p h t", h=2)
            # V natural per head with ones column: per key chunk kk, head h:
            # v_s[:, h, kk*65 : kk*65+64] = v values, col 64 = ones
            v_s = ppool.tile([128, 2 * nkc * 65], BF16, name="v_s")
            nc.vector.memset(v_s[:], 1.0)
            v3 = v_s[:].rearrange("p (h c) -> p h c", h=2)
            # unnormalized attention output (transposed) + sums row 64
            aoU0_s = ppool.tile([65, t], FP32R, name="aoU0_s")
            aoU1_s = ppool.tile([65, t], FP32R, name="aoU1_s")

            out_copy_flip = [0]

            def emit_epilogue(Q, po0, po1):
                """Per-query-chunk epilogue: normalize + output projection."""
                qsl = ts(Q, 512)
                # free the PV banks: copy (incl. sums row 64) to SBUF
                nc.vector.tensor_copy(aoU0_s[:, qsl], po0[:])
                nc.vector.tensor_copy(aoU1_s[:, qsl], po1[:])
                # broadcast sums row to 64 partitions (K=1 matmul), then
                # reciprocal on 64 lanes
                psb0 = ps_mi.tile([64, 512], FP32, tag="mi", name="psb0")
                nc.tensor.matmul(
                    psb0[:],
                    lhsT=ones_row[64:65, :],
                    rhs=aoU0_s[64:65, qsl],
                    start=True,
                    stop=True,
                )
                psb1 = ps_mi.tile([64, 512], FP32, tag="mi", name="psb1")
                nc.tensor.matmul(
                    psb1[:],
                    lhsT=ones_row[64:65, :],
                    rhs=aoU1_s[64:65, qsl],
                    start=True,
                    stop=True,
                )
                rbc0 = wpool.tile([64, 512], FP32, tag="bc", name="rbc0")
                nc.vector.reciprocal_approx_fast(rbc0[:], psb0[:])
                rbc1 = wpool.tile([64, 512], FP32, tag="bc", name="rbc1")
                nc.vector.reciprocal_approx_fast(rbc1[:], psb1[:])
                # normalized attention-out on GpSimd (SBUF-only engine);
                # head1 lands via an SBUF->SBUF DMA partition shift
                aoT_b = wpool.tile([128, 512], BF16, tag="ao", name="aoT_b")
                nc.gpsimd.tensor_mul(
                    aoT_b[0:64, :], aoU0_s[0:64, qsl].bitcast(FP32), rbc0[:]
                )
                aoT1 = wpool.tile([64, 512], BF16, tag="ao1", name="aoT1")
                nc.gpsimd.tensor_mul(
                    aoT1[:], aoU1_s[0:64, qsl].bitcast(FP32), rbc1[:]
                )
                nc.sync.dma_start(aoT_b[64:128, :], aoT1[:])
                for qq in range(4):
                    pso = ps_mi.tile([128, 512], FP32, tag="mi", name="pso")
                    nc.tensor.matmul(
                        pso[:],
                        lhsT=aoT_b[:, ts(qq, 128)],
                        rhs=woT_s[:],
                        start=True,
                        stop=True,
                    )
                    osb = wpool.tile([128, 512], FP32, tag="os", name="osb")
                    if out_copy_flip[0] % 2 == 0:
                        nc.scalar.copy(osb[:], pso[:])
                    else:
                        nc.vector.tensor_copy(osb[:], pso[:])
                    out_copy_flip[0] += 1
                    nc.sync.dma_start(outp[ds(Q * 512 + qq * 128, 128), :], osb[:])

            # ---- fused loop over 512-token super-chunks ----
            for tcx in range(nq):
                # -- QKV projection for tokens [tcx*512, (tcx+1)*512) --
                xts = []
                for d in range(4):
                    xt = xpool.tile([128, 512], BF16, tag=f"xt{d}", name=f"xt{d}")
                    nc.sync.dma_start(xt[:], xT[ts(d, 128), ts(tcx, 512)])
                    xts.append(xt)
                psqk = ps_sc.tile([128, 1024], FP32, tag="sc", name="psqk")
                for d in range(4):
                    nc.tensor.matmul(
                        psqk[:, 0:512],
                        lhsT=wq_s[:, ts(d, 128)],
                        rhs=xts[d][:],
                        start=(d == 0),
                        stop=(d == 3),
                    )
                for d in range(4):
                    nc.tensor.matmul(
                        psqk[:, 512:1024],
                        lhsT=wk_s[:, ts(d, 128)],
                        rhs=xts[d][:],
                        start=(d == 0),
                        stop=(d == 3),
                    )
                # one FD=1024 ScalarE copy: q half -> qkT[:,0,...], k half -> [:,1,...]
                nc.scalar.copy(
                    qkT3[:, :, ts(tcx, 512)],
                    psqk[:].rearrange("p (h n) -> p h n", h=2),
                )
                # V: one accumulating [128,512] tile (4 token sub-chunks x 4 d)
                psv = ps_mi.tile([128, 512], FP32, tag="mi", name="psv")
                for tt in range(4):
                    for d in range(4):
                        nc.tensor.matmul(
                            psv[:, ts(tt, 128)],
                            lhsT=xts[d][:, ts(tt, 128)],
                            rhs=wv_s[:, ts(d, 128)],
                            start=(d == 0),
                            stop=(d == 3),
                        )
                # one FD=512 DVE copy into both heads' v slots (ones col kept)
                nc.vector.tensor_copy(
                    v3[:, :, ds(tcx * 4 * 65, 4 * 65)].rearrange(
                        "p h (kk c) -> p h kk c", kk=4
                    )[:, :, :, 0:64],
                    psv[:]
                    .rearrange("p (tt h c) -> p h tt c", tt=4, h=2),
                )

                # -- attention for query chunk Q = tcx --
                Q = tcx
                po0 = ps_pv.tile([65, 512], FP32, tag="pv0", name="po0")
                po1 = ps_pv.tile([65, 512], FP32, tag="pv1", name="po1")
                nkq = 4 * Q + 4
                pts = {}
                last_scores = None
                # software-pipelined: scores/exp for chunk K are issued two
                # iterations ahead of the PV matmuls for chunk K-2
                for K in range(nkq + 2):
                    if K < nkq:
                        off = K * 128 - Q * 512
                        n0 = max(off, 0)
                        w = 512 - n0
                        pssc = ps_sc.tile([128, 1024], FP32, tag="sc", name="pssc")
                        nc.tensor.matmul(
                            pssc[:, n0:512],
                            lhsT=qkT3[0:64, 1, ts(K, 128)],
                            rhs=qkT3[0:64, 0, ds(Q * 512 + n0, w)],
                            start=True,
                            stop=True,
                        )
                        last_scores = nc.tensor.matmul(
                            pssc[:, 512 + n0 : 1024],
                            lhsT=qkT3[64:128, 1, ts(K, 128)],
                            rhs=qkT3[64:128, 0, ds(Q * 512 + n0, w)],
                            start=True,
                            stop=True,
                        )
                        pt = wpool.tile([128, 1024], BF16, tag="pt", name="pt", bufs=4)
                        src = pssc[:].rearrange("p (h n) -> p h n", h=2)[:, :, n0:512]
                        dst = pt[:].rearrange("p (h n) -> p h n", h=2)[:, :, n0:512]
                        diag = off >= 0
                        use_scalar = diag or (K % SCALAR_MOD) < SCALAR_SHARE
                        if use_scalar:
                            nc.scalar.activation(dst, src, AF.Exp, scale=inv_sqrt_d)
                        else:
                            # Schraudolph: bf16 bits of exp(x/8) via int16
                            nc.vector.tensor_scalar(
                                dst.bitcast(I16),
                                src,
                                EXP_A,
                                EXP_B,
                                mybir.AluOpType.mult,
                                mybir.AluOpType.add,
                            )
                        if diag:
                            # zero the not-yet-valid triangle (GpSimd)
                            nc.gpsimd.tensor_mul(
                                pt[:, ds(n0, 128)], pt[:, ds(n0, 128)], mask_s[:]
                            )
                            nc.gpsimd.tensor_mul(
                                pt[:, ds(512 + n0, 128)],
                                pt[:, ds(512 + n0, 128)],
                                mask_s[:],
                            )
                        pts[K] = (pt, n0, w)
                    if K >= 2:
                        Kp = K - 2
                        pt_p, n0_p, w_p = pts.pop(Kp)
                        st = Kp == 0
                        sp = Kp == nkq - 1
                        pv0_mm = nc.tensor.matmul(
                            po0[0:65, ds(n0_p, w_p)],
                            lhsT=v3[:, 0, ds(Kp * 65, 65)],
                            rhs=pt_p[:, ds(n0_p, w_p)],
                            start=st,
                            stop=sp,
                            skip_group_check=True,
                        )
                        if K < nkq and last_scores is not None:
                            # order-only edge: keep the PV pair AFTER the
                            # next chunk's scores on the PE queue
                            add_dep_helper(
                                pv0_mm.ins,
                                last_scores.ins,
                                sync=False,
                                reason="pipeline skew",
                            )
                        nc.tensor.matmul(
                            po1[0:65, ds(n0_p, w_p)],
                            lhsT=v3[:, 1, ds(Kp * 65, 65)],
                            rhs=pt_p[:, ds(512 + n0_p, w_p)],
                            start=st,
                            stop=sp,
                            skip_group_check=True,
                        )
                emit_epilogue(Q, po0, po1)
    nc.compile()
    return nc


def make_in_maps(x, W_QKV, W_O, t=T, n_cores=8):
    x = np.ascontiguousarray(np.asarray(x, dtype=np.float32))
    W_QKV = np.asarray(W_QKV, dtype=np.float32)
    W_O = np.asarray(W_O, dtype=np.float32)
    B = x.shape[0]
    bf16 = ml_dtypes.bfloat16
    xTs = [np.ascontiguousarray(x[b, :t].T).astype(bf16) for b in range(B)]
    in_maps = []
    for c in range(n_cores):
        b = c // 4
        g = c % 4
        hs = slice(2 * g * 64, 2 * g * 64 + 128)
        in_maps.append(
            {
                "xT": xTs[b],
                "wq": np.ascontiguousarray(W_QKV[0:512][hs].T).astype(bf16),
                "wk": np.ascontiguousarray(W_QKV[512:1024][hs].T).astype(bf16),
                "wv": np.ascontiguousarray(W_QKV[1024:1536][hs].T).astype(bf16),
                "woT": np.ascontiguousarray(W_O[:, hs].T).astype(bf16),
            }
        )
    return in_maps


def kernel(x, W_QKV, W_O):
    global LAST_RESULTS
    x = np.asarray(x, dtype=np.float32)
    B, t, _ = x.shape
    nc = build_program(t)
    in_maps = make_in_maps(x, W_QKV, W_O, t=t)
    res = run_bass_kernel_spmd(
        nc, in_maps, core_ids=list(range(8)), trace=TRACE
    )
    LAST_RESULTS = res
    parts = [r["outp"] for r in res.results]
    out = np.empty((B, t, DM), dtype=np.float32)
    for b in range(B):
        acc = np.zeros((t, DM), dtype=np.float64)
        for g in range(4):
            acc += parts[b * 4 + g]
        out[b] = acc.astype(np.float32)
    return out
